# revision 18
# baseline (speedup 1.0000x reference)
"""ContextAwareAttention Trainium2 kernel (v4).

Strategy (sized for the TimelineSim cost model):
  - Data-parallel over batch: B=128 -> 16 batches/core x 8 cores; SBATCH=2
    batches per superbatch ("SB") iteration.
  - fp8e4m3 DoubleRow matmuls (0.5 cyc/row) everywhere they pay:
    q/k/v projections, Wo, Wc1 (3-group Dekker), scores (q8/k8 repacked
    d-pairs via an SBUF->SBUF DMA), A@V (m-pairs = the two mc chunks of
    vt/pt), and the softmax denominator (ones lhsT over the same m-pairs).
  - mask * rel-pos-bias folded into the scores PSUM as ln(mask*expB) via an
    identity-matmul add (bf16 identity * 2^(kq+kk) x fp8 table): removes the
    DVE mask multiply entirely; exp() then emits P straight to fp8.
  - Per-token softmax normalization: denominator rows [2,256] -> DVE
    reciprocal -> bf16 -> broadcast to 128 partitions with a 2-row selector
    matmul -> ONE fused [128,256] multiply per (c,j).
  - DMAs merged (x/c/clo one slab, lnmb one slab, weights one slab, output
    one slab per SB) and spread over the SP + Pool(SWDGE) queues so no DMA
    queue paces the kernel.
  - LayerNorm: mu via DoubleRow matmul of the Wc1 row-sums; var from bf16
    squares; rstd = exp(-0.5*ln(512*var+eps')); pinned activation tables.
"""

import math

import numpy as np
import ml_dtypes

import concourse.bass as bass  # noqa: F401
import concourse.mybir as mybir
import concourse.tile as tile
from concourse import bacc
from concourse.bass_utils import run_bass_kernel_spmd

B, N, DIM, H, D = 128, 256, 512, 8, 64
N_CORES = 8
BPC = B // N_CORES            # 16
SBATCH = 2
NSUPER = BPC // SBATCH        # 8
SCALE = D ** -0.5
LN_EPS = 1e-5
F32 = mybir.dt.float32
BF16 = mybir.dt.bfloat16
F8 = mybir.dt.float8e4
NW = SBATCH * N               # 512
NP8 = np.dtype(ml_dtypes.float8_e4m3)
NPBF = np.dtype(ml_dtypes.bfloat16)

AF = mybir.ActivationFunctionType
DR = mybir.MatmulPerfMode.DoubleRow

OT_K = 5     # ot is stored as 2^OT_K * O/denom (fp8 range health)
MASKLN = -30.0  # ln(mask) stand-in for masked entries (exp -> ~1e-13)


def _emit(nc, tc, io, n_super, ks):
    (xccd, lnmbd, wcat, identd, wc2d, cvecd, outT) = io
    kq, kk, kv, kwo, kc1, ksum, guni, b1z, boz = ks
    g = OT_K + kwo
    kqk = kq + kk

    def mm(out, lhsT, rhs, start, stop, perf_mode=None):
        nc.tensor.matmul(out, lhsT, rhs, start=start, stop=stop,
                         perf_mode=perf_mode)

    with (
        nc.allow_low_precision(reason="fp8/bf16 design, verified vs oracle"),
        tc.tile_pool(name="consts", bufs=1) as consts,
        tc.tile_pool(name="io", bufs=3) as iop,
        tc.tile_pool(name="mb", bufs=2) as mbp,
        tc.tile_pool(name="work", bufs=2) as work,
        tc.tile_pool(name="pp", bufs=3) as ppool,
        tc.tile_pool(name="rows", bufs=2) as rows,
        tc.tile_pool(name="psum", bufs=2, space="PSUM") as psum,
    ):
        # ---- compile-time constants (no DMA) ----
        onecol = consts.tile([1, 128], BF16, name="onecol")
        nc.vector.memset(onecol, 1.0)
        negk1 = consts.tile([1, 128], BF16, name="negk1")
        nc.vector.memset(negk1, -(2.0 ** kc1))
        colones = consts.tile([128, 1], BF16, name="colones")
        nc.vector.memset(colones, 1.0)
        # denominator DR lhsT: value 2^(kv-OT_K) so ot = 2^OT_K * O exactly;
        # full 128-wide so the sum lands broadcast on every partition
        ones8dr = consts.tile([128, 2, 128], F8, name="ones8dr")
        nc.vector.memset(ones8dr, 2.0 ** (kv - OT_K))
        eps512 = consts.tile([1, 1], F32, name="eps512")
        nc.vector.memset(eps512, DIM * LN_EPS)
        lngln = consts.tile([1, 1], F32, name="lngln")
        nc.vector.memset(
            lngln, math.log(guni * math.sqrt(DIM)) if guni is not None else 0.0)

        # ---- DMA'd constants (q/k/v weights first: gate the first proj) ----
        w8 = consts.tile([128, 8, 4, 512], F8, name="w8")
        nc.sync.dma_start(
            out=w8[:, 0:3],
            in_=wcat[0:3].rearrange("t (kc p) f -> p t kc f", p=128))
        wq8, wk8, wv8, wo8 = w8[:, 0], w8[:, 1], w8[:, 2], w8[:, 3]
        w1a, w1b, w1c = w8[:, 4], w8[:, 5], w8[:, 6]
        w1st = w8[:, 7, :, 0:8]

        # sb0 inputs next, so their transfers aren't queued behind the
        # remaining constants on the shared DMA engines
        xcc0 = iop.tile([128, 3, 4, SBATCH, 256], F8, name="xcc0")
        nc.sync.dma_start(out=xcc0, in_=xccd[0])
        lnt0 = mbp.tile([128, SBATCH, 4, 2, 2, 256], F8, name="lnt0")
        nc.gpsimd.dma_start(
            out=lnt0,
            in_=lnmbd[0:SBATCH].rearrange("j p c h2 mc n -> p j c h2 mc n"))

        nc.scalar.dma_start(
            out=w8[:, 3:8],
            in_=wcat[3:8].rearrange("t (kc p) f -> p t kc f", p=128))
        ident = consts.tile([128, 128], BF16, name="ident")
        nc.scalar.dma_start(out=ident, in_=identd)
        wc2t = consts.tile([128, 4, 512], BF16, name="wc2t")
        nc.scalar.dma_start(out=wc2t,
                            in_=wc2d.rearrange("(kc p) f -> p kc f", p=128))
        cvt = consts.tile([128, 4, 4], F32, name="cvt")
        nc.scalar.dma_start(out=cvt,
                            in_=cvecd.rearrange("t (c p) -> p t c", p=128))
        lngc, lnbc, bocbc = cvt[:, 0], cvt[:, 1], cvt[:, 3]

        def issue_xcc(sb):
            xcc = iop.tile([128, 3, 4, SBATCH, 256], F8, name="xcc")
            nc.sync.dma_start(out=xcc, in_=xccd[sb])
            return xcc

        def issue_lnt(sb):
            lnt = mbp.tile([128, SBATCH, 4, 2, 2, 256], F8, name="lnt")
            nc.gpsimd.dma_start(
                out=lnt, in_=lnmbd[sb * SBATCH:(sb + 1) * SBATCH].rearrange(
                    "j p c h2 mc n -> p j c h2 mc n"))
            return lnt

        def qkprep(xcc_t):
            # q/k projections (fp8 DR -> fp8 evac), then repack into the
            # DR d-pair layout [32, r, qk, c, h2, j, n] via SBUF->SBUF DMA
            qkt = work.tile([128, 2, 4, SBATCH, 256], F8, name="qkt")
            for t, wsl in ((0, wq8), (1, wk8)):
                src = xcc_t[:, t]
                for c in range(4):
                    ps = psum.tile([128, NW], F32, tag="g", bufs=2)
                    for i in range(2):
                        mm(ps, wsl[:, 2 * i:2 * i + 2, c * 128:(c + 1) * 128],
                           src[:, 2 * i:2 * i + 2],
                           start=i == 0, stop=i == 1, perf_mode=DR)
                    nc.scalar.copy(
                        out=qkt[:, t, c].rearrange("p j n -> p (j n)"), in_=ps)
            qk8 = work.tile([32, 2, 2, 4, 2, SBATCH, 256], F8, name="qk8")
            for h2 in range(2):
                for r in range(2):
                    p0 = h2 * 64 + r * 32
                    nc.gpsimd.dma_start(out=qk8[:, r, :, :, h2],
                                        in_=qkt[p0:p0 + 32])
            return qk8

        carry = []
        carry2 = []
        # prologue: sb0's q/k pipeline (inputs already staged above)
        xcc_cur = xcc0
        lnt_cur = lnt0
        qk8_cur = qkprep(xcc_cur)
        for sb in range(n_super):
            b0 = sb * SBATCH
            last = sb == n_super - 1
            if not last:
                xcc_next = issue_xcc(sb + 1)
                lnt_next = issue_lnt(sb + 1)
            for fn in carry:
                fn()
            carry.clear()
            x8t, ct8, clo = xcc_cur[:, 0], xcc_cur[:, 1], xcc_cur[:, 2]
            lnt, qk8 = lnt_cur, qk8_cur

            # ---- v projection (token-major, fp8) ----
            vt = work.tile([128, SBATCH, 2, 512], F8, name="vt")
            for j in range(SBATCH):
                for mc in range(2):
                    ps = psum.tile([128, 512], F32, tag="g", bufs=2)
                    for i in range(2):
                        mm(ps, ct8[:, 2 * i:2 * i + 2, j,
                                   mc * 128:(mc + 1) * 128],
                           wv8[:, 2 * i:2 * i + 2], start=i == 0, stop=i == 1,
                           perf_mode=DR)
                    nc.vector.tensor_copy(out=vt[:, j, mc], in_=ps)

            for fn in carry2:
                fn()
            carry2.clear()

            # ---- LN stats closures ----
            state = {}

            def ln_mu():
                mu_ps = psum.tile([128, NW], F32, tag="g", bufs=2)
                for kc in range(4):
                    mm(mu_ps[0:1, :], w1st[:, kc, 0:1],
                       ct8[:, kc].rearrange("p j n -> p (j n)"),
                       start=kc == 0, stop=kc == 3)
                mu_r = rows.tile([1, NW], BF16, tag="r", bufs=6)
                nc.scalar.activation(out=mu_r, in_=mu_ps[0:1, :],
                                     func=AF.Identity,
                                     scale=2.0 ** (-ksum) / DIM)
                state["mu_r"] = mu_r

            def ln_var():
                sqt = state["sqt"]
                sq_ps = psum.tile([128, NW], F32, tag="g", bufs=2)
                for kc in range(4):
                    mm(sq_ps[0:1, :], colones, sqt[:, kc],
                       start=kc == 0, stop=kc == 3)
                ln_r = rows.tile([1, NW], F32, tag="r", bufs=6)
                nc.scalar.activation(out=ln_r, in_=sq_ps[0:1, :], func=AF.Ln,
                                     bias=eps512)
                a_r = rows.tile([1, NW], BF16, tag="r", bufs=6)
                nc.scalar.activation(out=a_r, in_=ln_r, func=AF.Exp,
                                     scale=-0.5, bias=lngln)
                ad_ps = psum.tile([128, NW], F32, tag="g", bufs=2)
                mm(ad_ps, onecol, a_r, start=True, stop=True)
                ad_sb = work.tile([128, NW], BF16, name="ad_sb")
                nc.vector.tensor_copy(out=ad_sb, in_=ad_ps)
                state["ad_sb"] = ad_sb

            ln_mu()

            # ---- context branch: h = c @ Wc1 + bc1, 3-group fp8 Dekker ----
            ht = work.tile([128, 4, NW], BF16, name="ht")
            sqt = work.tile([128, 4, NW], BF16, name="sqt")
            for c in range(4):
                ps = psum.tile([128, NW], F32, tag="g", bufs=2)
                cs = slice(c * 128, (c + 1) * 128)
                for i in range(2):
                    mm(ps, w1a[:, 2 * i:2 * i + 2, cs],
                       ct8[:, 2 * i:2 * i + 2], start=i == 0, stop=False,
                       perf_mode=DR)
                for i in range(2):
                    mm(ps, w1b[:, 2 * i:2 * i + 2, cs],
                       ct8[:, 2 * i:2 * i + 2], start=False, stop=False,
                       perf_mode=DR)
                for i in range(2):
                    mm(ps, w1c[:, 2 * i:2 * i + 2, cs],
                       clo[:, 2 * i:2 * i + 2], start=False, stop=False,
                       perf_mode=DR)
                mm(ps, negk1, state["mu_r"], start=False, stop=True)
                nc.scalar.activation(out=ht[:, c], in_=ps, func=AF.Identity,
                                     scale=2.0 ** (-kc1))
                nc.vector.tensor_mul(out=sqt[:, c], in0=ht[:, c],
                                     in1=ht[:, c])
            state["sqt"] = sqt

            # ---- attention + interleaved LN-normalize / ctx2+out1 ----
            ot = work.tile([128, 4, SBATCH, 256], F8, name="ot")
            res = iop.tile([128, SBATCH, 4, 256], BF16, name="res")

            def attn1(c, j):
                s_ps = psum.tile([128, 2, NW], F32, tag="s", bufs=2)
                pt8 = ppool.tile([128, 2, 2, 256], F8, tag="p", name="pt8",
                                 bufs=8)
                for h2 in range(2):
                    # ln(mask*expB) add: identity(*2^kqk) @ fp8 table
                    mm(s_ps[:, h2], ident, lnt[:, j, c, h2],
                       start=True, stop=False)
                    for mc in range(2):
                        mm(s_ps[:, h2, mc * 256:(mc + 1) * 256],
                           qk8[:, :, 1, c, h2, j, mc * 128:(mc + 1) * 128],
                           qk8[:, :, 0, c, h2, j],
                           start=False, stop=mc == 1, perf_mode=DR)
                    # per-h2 exp releases P (and the s bank) early
                    nc.scalar.activation(
                        out=pt8[:, :, h2, :],
                        in_=s_ps[:, h2].rearrange("p (mc n) -> p mc n", mc=2),
                        func=AF.Exp, scale=2.0 ** (-kqk))
                return s_ps, pt8

            def attn2(c, j, s_ps, pt8):
                # denominators, broadcast on all 128 parts, in a g-ring bank
                # (s_ps's lifetime ends at exp, freeing the s ring sooner)
                dn = psum.tile([128, NW], F32, tag="g", bufs=2)
                for h2 in range(2):
                    mm(dn[:, h2 * 256:(h2 + 1) * 256], ones8dr,
                       pt8[:, :, h2], start=True, stop=True, perf_mode=DR)
                oo = psum.tile([64, 2, 256], F32, tag="oo", bufs=2)
                for h2 in range(2):
                    hd = (2 * c + h2) * 64
                    mm(oo[:, h2], vt[:, j, :, hd:hd + 64], pt8[:, :, h2],
                       start=True, stop=True, perf_mode=DR)
                recd = rows.tile([64, 2, 256], F32, tag="rb", bufs=6)
                nc.vector.reciprocal(out=recd, in_=dn[0:64, :])
                for h2 in range(2):
                    nc.vector.tensor_mul(
                        out=ot[h2 * 64:(h2 + 1) * 64, c, j],
                        in0=oo[:, h2], in1=recd[:, h2])

            def normalize2(cp):
                # rl = relu((h-mu)*rstd), in place (g uniform; rstd>0)
                ad_sb = state["ad_sb"]
                sl = ht[:, 2 * cp:2 * cp + 2]
                nc.vector.tensor_mul(
                    out=sl, in0=sl,
                    in1=ad_sb.unsqueeze(1).broadcast_to([128, 2, NW]))
                nc.vector.tensor_scalar_max(
                    out=sl.rearrange("p c w -> p (c w)"),
                    in0=sl.rearrange("p c w -> p (c w)"), scalar1=0.0)

            def normalize(c):
                ad_sb = state["ad_sb"]
                nc.vector.tensor_mul(out=ht[:, c], in0=ht[:, c], in1=ad_sb)
                nc.scalar.activation(out=ht[:, c], in_=ht[:, c],
                                     func=AF.Relu,
                                     scale=lngc[:, c:c + 1],
                                     bias=lnbc[:, c:c + 1])

            def ctx2wo(j, fp, ht=ht, ot=ot, res=res, b0=b0):
                co = psum.tile([128, NW], F32, tag="g", bufs=2)
                for fl in range(2):
                    f = 2 * fp + fl
                    dst = co[:, fl * 256:(fl + 1) * 256]
                    for kc in range(4):
                        mm(dst, wc2t[:, kc, f * 128:(f + 1) * 128],
                           ht[:, kc, j * 256:(j + 1) * 256],
                           start=kc == 0, stop=False)
                    for i in range(2):
                        mm(dst,
                           wo8[:, 2 * i:2 * i + 2, f * 128:(f + 1) * 128],
                           ot[:, 2 * i:2 * i + 2, j],
                           start=False, stop=i == 1, perf_mode=DR)
                if boz:
                    if j == 0:
                        nc.scalar.activation(
                            out=res[:, j, 2 * fp:2 * fp + 2, :],
                            in_=co.rearrange("p (f n) -> p f n", f=2),
                            func=AF.Identity, scale=2.0 ** (-g))
                    else:
                        nc.vector.tensor_scalar_mul(
                            out=res[:, j, 2 * fp:2 * fp + 2, :],
                            in0=co.rearrange("p (f n) -> p f n", f=2),
                            scalar1=2.0 ** (-g))
                else:
                    for fl in range(2):
                        f = 2 * fp + fl
                        nc.scalar.activation(
                            out=res[:, j, f, :],
                            in_=co[:, fl * 256:(fl + 1) * 256],
                            func=AF.Identity, scale=2.0 ** (-g),
                            bias=bocbc[:, f:f + 1])
                if j == 1 and fp == 1:
                    carry.append(
                        lambda res=res, b0=b0: nc.sync.dma_start(
                            out=outT[b0:b0 + SBATCH].rearrange(
                                "j (c p) n -> p j c n", p=128),
                            in_=res))

            # software-pipelined emission; next SB's q/k pipeline is emitted
            # mid-attention so its repack DMA latency is fully hidden
            p00 = attn1(0, 0)
            p10 = attn1(1, 0)
            ln_var()
            p20 = attn1(2, 0)
            attn2(0, 0, *p00)
            if guni is not None:
                normalize2(0)
            else:
                normalize(0)
                normalize(1)
            p30 = attn1(3, 0)
            attn2(1, 0, *p10)
            if guni is not None:
                normalize2(1)
            else:
                normalize(2)
                normalize(3)
            if not last:
                qk8_next = qkprep(xcc_next)
            p01 = attn1(0, 1)
            attn2(2, 0, *p20)
            p11 = attn1(1, 1)
            attn2(3, 0, *p30)
            ctx2wo(0, 0)
            p21 = attn1(2, 1)
            attn2(0, 1, *p01)
            ctx2wo(0, 1)
            p31 = attn1(3, 1)
            attn2(1, 1, *p11)
            attn2(2, 1, *p21)
            attn2(3, 1, *p31)
            if last:
                ctx2wo(1, 0)
                ctx2wo(1, 1)
            else:
                carry2.append(lambda f=ctx2wo: f(1, 0))
                carry2.append(lambda f=ctx2wo: f(1, 1))
                xcc_cur, lnt_cur, qk8_cur = xcc_next, lnt_next, qk8_next
        for fn in carry:
            fn()


def build(n_super, ks):
    # Pin the activation table (see baseline): only natural_log_exp_and_others
    import concourse.bacc as bacc_mod
    from concourse.hw_specs import get_activation_tables as _gat

    def pinned_tables(arch):
        tabs = _gat(arch)
        return {name: (s if name == "natural_log_exp_and_others" else set())
                for name, s in tabs.items()}

    nc = bacc.Bacc("TRN2", target_bir_lowering=False, debug=False,
                   num_devices=N_CORES)
    dt = nc.dram_tensor
    io = (
        dt("xcc", [NSUPER, 128, 3, 4, SBATCH, N], F8,
           kind="ExternalInput").ap(),
        dt("lnmb", [BPC, 128, 4, 2, 2, N], F8, kind="ExternalInput").ap(),
        dt("wcat", [8, DIM, DIM], F8, kind="ExternalInput").ap(),
        dt("ident", [128, 128], BF16, kind="ExternalInput").ap(),
        dt("wc2", [DIM, DIM], BF16, kind="ExternalInput").ap(),
        dt("cvec", [4, DIM], F32, kind="ExternalInput").ap(),
        dt("outT", [BPC, DIM, N], BF16, kind="ExternalOutput").ap(),
    )
    with tile.TileContext(nc) as tc:
        _emit(nc, tc, io, n_super, ks)
    saved = bacc_mod.get_activation_tables
    bacc_mod.get_activation_tables = pinned_tables
    try:
        nc.compile()
    finally:
        bacc_mod.get_activation_tables = saved
    return nc


def _k_of(absmax, target=120.0):
    return int(math.floor(math.log2(target / max(absmax, 1e-30))))


def prep_in_maps(x, context, mask, Wq, Wk, Wv, Wc1, bc1, ln_g, ln_b, Wc2, bc2,
                 Wo, bo, bias_table, rel_index):
    f = np.float32
    x = np.asarray(x, f)
    context = np.asarray(context, f)
    mask = np.asarray(mask)
    Wq = np.asarray(Wq, f) * SCALE
    Wk = np.asarray(Wk, f)
    Wv = np.asarray(Wv, f)
    Wo = np.asarray(Wo, f)
    Wc1 = np.asarray(Wc1, f)
    Wc2 = np.asarray(Wc2, f)

    # q/k/v weights scaled 4 bits lower so fp8 activations (~sigma 29) stay
    # well inside IEEE e4m3 range (max finite 240)
    kq = _k_of(np.abs(Wq).max(), 7.5)
    kk = _k_of(np.abs(Wk).max(), 7.5)
    kv = _k_of(np.abs(Wv).max(), 7.5)
    kwo = _k_of(np.abs(Wo).max())
    kc1 = _k_of(np.abs(Wc1).max())
    w1sum = Wc1.sum(axis=1, keepdims=True)
    ksum = _k_of(np.abs(w1sum).max())
    lng_a = np.asarray(ln_g, f)
    lnb_a = np.asarray(ln_b, f)
    guni = (float(lng_a[0]) if np.all(lng_a == lng_a[0]) and float(lng_a[0]) > 0
            and np.all(lnb_a == 0.0) else None)
    b1z = bool(np.all(np.asarray(bc1, f) == 0.0))
    boz = bool(np.all(np.asarray(bo, f) == 0.0)
               and np.all(np.asarray(bc2, f) == 0.0))
    ks = (kq, kk, kv, kwo, kc1, ksum, guni, b1z, boz)
    g = OT_K + kwo

    xT = np.ascontiguousarray(
        x.reshape(N_CORES, BPC, N, DIM).transpose(0, 1, 3, 2))
    cT = np.ascontiguousarray(
        context.reshape(N_CORES, BPC, N, DIM).transpose(0, 1, 3, 2))
    x8 = xT.astype(NP8)
    c8 = cT.astype(NP8)
    clo = ((cT - c8.astype(f)) * 32.0).astype(NP8)

    def slab(a):
        # [cr, BPC, 512, 256] -> [cr, NSUPER, 128(p), 4(kc), SBATCH(j), 256]
        a = a.reshape(N_CORES, NSUPER, SBATCH, 4, 128, N)
        return a.transpose(0, 1, 4, 3, 2, 5)

    # one slab: [cr, NSUPER, 128, 3(x/c/clo), 4, SBATCH, 256]
    xcc = np.ascontiguousarray(
        np.stack([slab(x8), slab(c8), slab(clo)], axis=3))

    # ln(mask * expB):  [cr, b, 128(p), 4(c), 2(h2), 2(mc), 256(n)] fp8
    lnbias = np.asarray(bias_table, f)[np.asarray(rel_index)].transpose(2, 1, 0)
    mT = mask.reshape(N_CORES, BPC, N, N).transpose(0, 1, 3, 2)
    lnmb = np.where(mT[:, :, None, :, :] == 0, np.float32(MASKLN),
                    lnbias[None, None].astype(f))
    lnmb = lnmb.reshape(N_CORES, BPC, 4, 2, 2, 128, N).transpose(
        0, 1, 5, 2, 3, 4, 6)
    lnmb = np.ascontiguousarray(lnmb).astype(NP8)

    w1as = Wc1 * 2.0 ** kc1
    w1a = w1as.astype(NP8)
    w1b = (w1as - w1a.astype(f)).astype(NP8)
    w1c = (w1as / 32.0).astype(NP8)
    w1s_pad = np.zeros((DIM, DIM), f)
    w1s_pad[:, 0:8] = np.repeat(w1sum * 2.0 ** ksum, 8, axis=1)

    wcat = np.ascontiguousarray(np.stack([
        (Wq * 2.0 ** kq).astype(NP8),
        (Wk * 2.0 ** kk).astype(NP8),
        (Wv * 2.0 ** kv).astype(NP8),
        (Wo * 2.0 ** kwo).astype(NP8),
        w1a, w1b, w1c,
        w1s_pad.astype(NP8),
    ]))

    ident = np.ascontiguousarray(
        (np.eye(128, dtype=f) * 2.0 ** (kq + kk)).astype(NPBF))
    cvec = np.ascontiguousarray(np.stack([
        np.asarray(ln_g, f) * math.sqrt(DIM),
        np.asarray(ln_b, f),
        np.asarray(bc1, f),
        np.asarray(bo, f) + np.asarray(bc2, f),
    ]))

    shared = dict(
        wcat=wcat,
        ident=ident,
        wc2=np.ascontiguousarray(Wc2 * 2.0 ** g).astype(NPBF),
        cvec=cvec,
    )
    in_maps = [dict(xcc=xcc[c], lnmb=lnmb[c], **shared)
               for c in range(N_CORES)]
    return in_maps, ks


_nc_cache = {}


def _get_nc(n_super, ks):
    key = (n_super, ks)
    if key not in _nc_cache:
        _nc_cache[key] = build(n_super, ks)
    return _nc_cache[key]


def assemble_out(results):
    outT = np.stack([np.asarray(results[c]["outT"]).astype(np.float32)
                     for c in range(N_CORES)])
    return np.ascontiguousarray(
        outT.transpose(0, 1, 3, 2).reshape(B, N, DIM))


def kernel(**inputs):
    in_maps, ks = prep_in_maps(**inputs)
    nc = _get_nc(NSUPER, ks)
    res = run_bass_kernel_spmd(nc, in_maps, core_ids=list(range(N_CORES)))
    return assemble_out(res.results)


# revision 19
# speedup vs baseline: 1.1181x; 1.1181x over previous
"""ContextAwareAttention Trainium2 kernel (v4).

Strategy (sized for the TimelineSim cost model):
  - Data-parallel over batch: B=128 -> 16 batches/core x 8 cores; SBATCH=2
    batches per superbatch ("SB") iteration.
  - fp8e4m3 DoubleRow matmuls (0.5 cyc/row) everywhere they pay:
    q/k/v projections, Wo, Wc1 (3-group Dekker), scores (q8/k8 repacked
    d-pairs via an SBUF->SBUF DMA), A@V (m-pairs = the two mc chunks of
    vt/pt), and the softmax denominator (ones lhsT over the same m-pairs).
  - mask * rel-pos-bias folded into the scores PSUM as ln(mask*expB) via an
    identity-matmul add (bf16 identity * 2^(kq+kk) x fp8 table): removes the
    DVE mask multiply entirely; exp() then emits P straight to fp8.
  - Per-token softmax normalization: denominator rows [2,256] -> DVE
    reciprocal -> bf16 -> broadcast to 128 partitions with a 2-row selector
    matmul -> ONE fused [128,256] multiply per (c,j).
  - DMAs merged (x/c/clo one slab, lnmb one slab, weights one slab, output
    one slab per SB) and spread over the SP + Pool(SWDGE) queues so no DMA
    queue paces the kernel.
  - LayerNorm: mu via DoubleRow matmul of the Wc1 row-sums; var from bf16
    squares; rstd = exp(-0.5*ln(512*var+eps')); pinned activation tables.
"""

import math

import numpy as np
import ml_dtypes

import concourse.bass as bass  # noqa: F401
import concourse.mybir as mybir
import concourse.tile as tile
from concourse import bacc
from concourse.bass_utils import run_bass_kernel_spmd

B, N, DIM, H, D = 128, 256, 512, 8, 64
N_CORES = 8
BPC = B // N_CORES            # 16
SBATCH = 2
NSUPER = BPC // SBATCH        # 8
SCALE = D ** -0.5
LN_EPS = 1e-5
F32 = mybir.dt.float32
BF16 = mybir.dt.bfloat16
F8 = mybir.dt.float8e4
NW = SBATCH * N               # 512
NP8 = np.dtype(ml_dtypes.float8_e4m3)
NPBF = np.dtype(ml_dtypes.bfloat16)

AF = mybir.ActivationFunctionType
DR = mybir.MatmulPerfMode.DoubleRow

OT_K = 5     # ot is stored as 2^OT_K * O/denom (fp8 range health)
MASKLN = -30.0  # ln(mask) stand-in for masked entries (exp -> ~1e-13)


def _emit(nc, tc, io, n_super, ks):
    (xccd, lnmbd, wcat, identd, wc2d, cvecd, outT) = io
    kq, kk, kv, kwo, kc1, ksum, guni, b1z, boz = ks
    g = OT_K + kwo
    kqk = kq + kk

    def mm(out, lhsT, rhs, start, stop, perf_mode=None):
        nc.tensor.matmul(out, lhsT, rhs, start=start, stop=stop,
                         perf_mode=perf_mode)

    with (
        nc.allow_low_precision(reason="fp8/bf16 design, verified vs oracle"),
        tc.tile_pool(name="consts", bufs=1) as consts,
        tc.tile_pool(name="io", bufs=3) as iop,
        tc.tile_pool(name="mb", bufs=2) as mbp,
        tc.tile_pool(name="work", bufs=2) as work,
        tc.tile_pool(name="pp", bufs=3) as ppool,
        tc.tile_pool(name="rows", bufs=2) as rows,
        tc.tile_pool(name="psum", bufs=2, space="PSUM") as psum,
    ):
        # ---- compile-time constants (no DMA) ----
        onecol = consts.tile([1, 128], BF16, name="onecol")
        nc.vector.memset(onecol, 1.0)
        negk1 = consts.tile([1, 128], BF16, name="negk1")
        nc.vector.memset(negk1, -(2.0 ** kc1))
        colones = consts.tile([128, 1], BF16, name="colones")
        nc.vector.memset(colones, 1.0)
        # denominator DR lhsT: value 2^(kv-OT_K) so ot = 2^OT_K * O exactly;
        # full 128-wide so the sum lands broadcast on every partition
        ones8dr = consts.tile([128, 2, 128], F8, name="ones8dr")
        nc.vector.memset(ones8dr, 2.0 ** (kv - OT_K))
        eps512 = consts.tile([1, 1], F32, name="eps512")
        nc.vector.memset(eps512, DIM * LN_EPS)
        lngln = consts.tile([1, 1], F32, name="lngln")
        nc.vector.memset(
            lngln, math.log(guni * math.sqrt(DIM)) if guni is not None else 0.0)

        # ---- DMA'd constants (q/k/v weights first: gate the first proj) ----
        w8 = consts.tile([128, 8, 4, 512], F8, name="w8")
        nc.sync.dma_start(
            out=w8[:, 0:3],
            in_=wcat[0:3].rearrange("t (kc p) f -> p t kc f", p=128))
        wq8, wk8, wv8, wo8 = w8[:, 0], w8[:, 1], w8[:, 2], w8[:, 3]
        w1a, w1b, w1c = w8[:, 4], w8[:, 5], w8[:, 6]
        w1st = w8[:, 7, :, 0:8]

        # sb0 inputs next, so their transfers aren't queued behind the
        # remaining constants on the shared DMA engines
        xcc0 = iop.tile([128, 3, 4, SBATCH, 256], F8, name="xcc0")
        nc.sync.dma_start(out=xcc0, in_=xccd[0])
        lnt0 = mbp.tile([128, SBATCH, 4, 2, 2, 256], F8, name="lnt0")
        nc.gpsimd.dma_start(
            out=lnt0,
            in_=lnmbd[0:SBATCH].rearrange("j p c h2 mc n -> p j c h2 mc n"))

        nc.scalar.dma_start(
            out=w8[:, 3:8],
            in_=wcat[3:8].rearrange("t (kc p) f -> p t kc f", p=128))
        ident = consts.tile([128, 128], BF16, name="ident")
        nc.scalar.dma_start(out=ident, in_=identd)
        wc2t = consts.tile([128, 4, 512], BF16, name="wc2t")
        nc.scalar.dma_start(out=wc2t,
                            in_=wc2d.rearrange("(kc p) f -> p kc f", p=128))
        cvt = consts.tile([128, 4, 4], F32, name="cvt")
        nc.scalar.dma_start(out=cvt,
                            in_=cvecd.rearrange("t (c p) -> p t c", p=128))
        lngc, lnbc, bocbc = cvt[:, 0], cvt[:, 1], cvt[:, 3]

        def issue_xcc(sb):
            xcc = iop.tile([128, 3, 4, SBATCH, 256], F8, name="xcc")
            nc.sync.dma_start(out=xcc, in_=xccd[sb])
            return xcc

        def issue_lnt(sb):
            lnt = mbp.tile([128, SBATCH, 4, 2, 2, 256], F8, name="lnt")
            nc.gpsimd.dma_start(
                out=lnt, in_=lnmbd[sb * SBATCH:(sb + 1) * SBATCH].rearrange(
                    "j p c h2 mc n -> p j c h2 mc n"))
            return lnt

        def qkprep(xcc_t):
            # q/k projections (fp8 DR -> fp8 evac), then repack into the
            # DR d-pair layout [32, r, qk, c, h2, j, n] via SBUF->SBUF DMA
            qkt = work.tile([128, 2, 4, SBATCH, 256], F8, name="qkt")
            for t, wsl in ((0, wq8), (1, wk8)):
                src = xcc_t[:, t]
                for c in range(4):
                    ps = psum.tile([128, NW], F32, tag="g", bufs=2)
                    for i in range(2):
                        mm(ps, wsl[:, 2 * i:2 * i + 2, c * 128:(c + 1) * 128],
                           src[:, 2 * i:2 * i + 2],
                           start=i == 0, stop=i == 1, perf_mode=DR)
                    nc.scalar.copy(
                        out=qkt[:, t, c].rearrange("p j n -> p (j n)"), in_=ps)
            qk8 = work.tile([32, 2, 2, 4, 2, SBATCH, 256], F8, name="qk8")
            for h2 in range(2):
                for r in range(2):
                    p0 = h2 * 64 + r * 32
                    nc.gpsimd.dma_start(out=qk8[:, r, :, :, h2],
                                        in_=qkt[p0:p0 + 32])
            return qk8

        carry = []
        carry2 = []
        # prologue: sb0's q/k pipeline (inputs already staged above)
        xcc_cur = xcc0
        lnt_cur = lnt0
        qk8_cur = qkprep(xcc_cur)
        for sb in range(n_super):
            b0 = sb * SBATCH
            last = sb == n_super - 1
            if not last:
                xcc_next = issue_xcc(sb + 1)
                lnt_next = issue_lnt(sb + 1)
            for fn in carry:
                fn()
            carry.clear()
            x8t, ct8, clo = xcc_cur[:, 0], xcc_cur[:, 1], xcc_cur[:, 2]
            lnt, qk8 = lnt_cur, qk8_cur

            # ---- v projection (token-major, fp8) ----
            vt = work.tile([128, SBATCH, 2, 512], F8, name="vt")
            for j in range(SBATCH):
                for mc in range(2):
                    ps = psum.tile([128, 512], F32, tag="g", bufs=2)
                    for i in range(2):
                        mm(ps, ct8[:, 2 * i:2 * i + 2, j,
                                   mc * 128:(mc + 1) * 128],
                           wv8[:, 2 * i:2 * i + 2], start=i == 0, stop=i == 1,
                           perf_mode=DR)
                    nc.vector.tensor_copy(out=vt[:, j, mc], in_=ps)

            for fn in carry2:
                fn()
            carry2.clear()

            # ---- LN stats closures ----
            state = {}

            def ln_mu():
                mu_ps = psum.tile([128, NW], F32, tag="g", bufs=2)
                for kc in range(4):
                    mm(mu_ps[0:1, :], w1st[:, kc, 0:1],
                       ct8[:, kc].rearrange("p j n -> p (j n)"),
                       start=kc == 0, stop=kc == 3)
                mu_r = rows.tile([1, NW], BF16, tag="r", bufs=6)
                nc.scalar.activation(out=mu_r, in_=mu_ps[0:1, :],
                                     func=AF.Identity,
                                     scale=2.0 ** (-ksum) / DIM)
                state["mu_r"] = mu_r

            def ln_var():
                sqt = state["sqt"]
                sq_ps = psum.tile([128, NW], F32, tag="g", bufs=2)
                for kc in range(4):
                    mm(sq_ps[0:1, :], colones, sqt[:, kc],
                       start=kc == 0, stop=kc == 3)
                ln_r = rows.tile([1, NW], F32, tag="r", bufs=6)
                nc.scalar.activation(out=ln_r, in_=sq_ps[0:1, :], func=AF.Ln,
                                     bias=eps512)
                a_r = rows.tile([1, NW], BF16, tag="r", bufs=6)
                nc.scalar.activation(out=a_r, in_=ln_r, func=AF.Exp,
                                     scale=-0.5, bias=lngln)
                ad_ps = psum.tile([128, NW], F32, tag="g", bufs=2)
                mm(ad_ps, onecol, a_r, start=True, stop=True)
                ad_sb = work.tile([128, NW], BF16, name="ad_sb")
                nc.vector.tensor_copy(out=ad_sb, in_=ad_ps)
                state["ad_sb"] = ad_sb

            ln_mu()

            # ---- context branch: h = c @ Wc1 + bc1, 3-group fp8 Dekker ----
            ht = work.tile([128, 4, NW], BF16, name="ht")
            sqt = work.tile([128, 4, NW], BF16, name="sqt")
            for c in range(4):
                ps = psum.tile([128, NW], F32, tag="g", bufs=2)
                cs = slice(c * 128, (c + 1) * 128)
                for i in range(2):
                    mm(ps, w1a[:, 2 * i:2 * i + 2, cs],
                       ct8[:, 2 * i:2 * i + 2], start=i == 0, stop=False,
                       perf_mode=DR)
                for i in range(2):
                    mm(ps, w1b[:, 2 * i:2 * i + 2, cs],
                       ct8[:, 2 * i:2 * i + 2], start=False, stop=False,
                       perf_mode=DR)
                for i in range(2):
                    mm(ps, w1c[:, 2 * i:2 * i + 2, cs],
                       clo[:, 2 * i:2 * i + 2], start=False, stop=False,
                       perf_mode=DR)
                mm(ps, negk1, state["mu_r"], start=False, stop=True)
                nc.scalar.activation(out=ht[:, c], in_=ps, func=AF.Identity,
                                     scale=2.0 ** (-kc1))
                nc.vector.tensor_mul(out=sqt[:, c], in0=ht[:, c],
                                     in1=ht[:, c])
            state["sqt"] = sqt

            # ---- attention + interleaved LN-normalize / ctx2+out1 ----
            ot = work.tile([128, 4, SBATCH, 256], F8, name="ot")
            res = iop.tile([128, SBATCH, 4, 256], BF16, name="res")

            def attn1(c, j):
                s_ps = psum.tile([128, 2, NW], F32, tag="s", bufs=2)
                pt8 = ppool.tile([128, 2, 2, 256], F8, tag="p", name="pt8",
                                 bufs=8)
                for h2 in range(2):
                    # ln(mask*expB) add: identity(*2^kqk) @ fp8 table
                    mm(s_ps[:, h2], ident, lnt[:, j, c, h2],
                       start=True, stop=False)
                    for mc in range(2):
                        mm(s_ps[:, h2, mc * 256:(mc + 1) * 256],
                           qk8[:, :, 1, c, h2, j, mc * 128:(mc + 1) * 128],
                           qk8[:, :, 0, c, h2, j],
                           start=False, stop=mc == 1, perf_mode=DR)
                    # per-h2 exp releases P (and the s bank) early
                    nc.scalar.activation(
                        out=pt8[:, :, h2, :],
                        in_=s_ps[:, h2].rearrange("p (mc n) -> p mc n", mc=2),
                        func=AF.Exp, scale=2.0 ** (-kqk))
                return s_ps, pt8

            def attn2(c, j, s_ps, pt8):
                # denominators, broadcast on all 128 parts of dead scores PSUM
                for h2 in range(2):
                    mm(s_ps[:, 1, h2 * 256:(h2 + 1) * 256], ones8dr,
                       pt8[:, :, h2], start=True, stop=True, perf_mode=DR)
                oo = psum.tile([64, 2, 256], F32, tag="oo", bufs=2)
                for h2 in range(2):
                    hd = (2 * c + h2) * 64
                    mm(oo[:, h2], vt[:, j, :, hd:hd + 64], pt8[:, :, h2],
                       start=True, stop=True, perf_mode=DR)
                recd = rows.tile([64, 2, 256], F32, tag="rb", bufs=6)
                nc.vector.reciprocal(out=recd, in_=s_ps[0:64, 1, :])
                for h2 in range(2):
                    nc.vector.tensor_mul(
                        out=ot[h2 * 64:(h2 + 1) * 64, c, j],
                        in0=oo[:, h2], in1=recd[:, h2])

            def normalize2(cp):
                # rl = relu((h-mu)*rstd), in place (g uniform; rstd>0)
                ad_sb = state["ad_sb"]
                sl = ht[:, 2 * cp:2 * cp + 2]
                nc.vector.tensor_mul(
                    out=sl, in0=sl,
                    in1=ad_sb.unsqueeze(1).broadcast_to([128, 2, NW]))
                nc.vector.tensor_scalar_max(
                    out=sl.rearrange("p c w -> p (c w)"),
                    in0=sl.rearrange("p c w -> p (c w)"), scalar1=0.0)

            def normalize(c):
                ad_sb = state["ad_sb"]
                nc.vector.tensor_mul(out=ht[:, c], in0=ht[:, c], in1=ad_sb)
                nc.scalar.activation(out=ht[:, c], in_=ht[:, c],
                                     func=AF.Relu,
                                     scale=lngc[:, c:c + 1],
                                     bias=lnbc[:, c:c + 1])

            def ctx2wo(j, fp, ht=ht, ot=ot, res=res, b0=b0):
                co = psum.tile([128, NW], F32, tag="g", bufs=2)
                for fl in range(2):
                    f = 2 * fp + fl
                    dst = co[:, fl * 256:(fl + 1) * 256]
                    for kc in range(4):
                        mm(dst, wc2t[:, kc, f * 128:(f + 1) * 128],
                           ht[:, kc, j * 256:(j + 1) * 256],
                           start=kc == 0, stop=False)
                    for i in range(2):
                        mm(dst,
                           wo8[:, 2 * i:2 * i + 2, f * 128:(f + 1) * 128],
                           ot[:, 2 * i:2 * i + 2, j],
                           start=False, stop=i == 1, perf_mode=DR)
                if boz:
                    if j == 0:
                        nc.scalar.activation(
                            out=res[:, j, 2 * fp:2 * fp + 2, :],
                            in_=co.rearrange("p (f n) -> p f n", f=2),
                            func=AF.Identity, scale=2.0 ** (-g))
                    else:
                        nc.vector.tensor_scalar_mul(
                            out=res[:, j, 2 * fp:2 * fp + 2, :],
                            in0=co.rearrange("p (f n) -> p f n", f=2),
                            scalar1=2.0 ** (-g))
                else:
                    for fl in range(2):
                        f = 2 * fp + fl
                        nc.scalar.activation(
                            out=res[:, j, f, :],
                            in_=co[:, fl * 256:(fl + 1) * 256],
                            func=AF.Identity, scale=2.0 ** (-g),
                            bias=bocbc[:, f:f + 1])
                if j == 1 and fp == 1:
                    carry.append(
                        lambda res=res, b0=b0: nc.sync.dma_start(
                            out=outT[b0:b0 + SBATCH].rearrange(
                                "j (c p) n -> p j c n", p=128),
                            in_=res))

            # software-pipelined emission; next SB's q/k pipeline is emitted
            # mid-attention so its repack DMA latency is fully hidden
            p00 = attn1(0, 0)
            p10 = attn1(1, 0)
            ln_var()
            p20 = attn1(2, 0)
            attn2(0, 0, *p00)
            if guni is not None:
                normalize2(0)
            else:
                normalize(0)
                normalize(1)
            p30 = attn1(3, 0)
            attn2(1, 0, *p10)
            if guni is not None:
                normalize2(1)
            else:
                normalize(2)
                normalize(3)
            if not last:
                qk8_next = qkprep(xcc_next)
            p01 = attn1(0, 1)
            attn2(2, 0, *p20)
            p11 = attn1(1, 1)
            attn2(3, 0, *p30)
            ctx2wo(0, 0)
            p21 = attn1(2, 1)
            attn2(0, 1, *p01)
            ctx2wo(0, 1)
            p31 = attn1(3, 1)
            attn2(1, 1, *p11)
            attn2(2, 1, *p21)
            attn2(3, 1, *p31)
            if last:
                ctx2wo(1, 0)
                ctx2wo(1, 1)
            else:
                carry2.append(lambda f=ctx2wo: f(1, 0))
                carry2.append(lambda f=ctx2wo: f(1, 1))
                xcc_cur, lnt_cur, qk8_cur = xcc_next, lnt_next, qk8_next
        for fn in carry:
            fn()


def build(n_super, ks):
    # Pin the activation table (see baseline): only natural_log_exp_and_others
    import concourse.bacc as bacc_mod
    from concourse.hw_specs import get_activation_tables as _gat

    def pinned_tables(arch):
        tabs = _gat(arch)
        return {name: (s if name == "natural_log_exp_and_others" else set())
                for name, s in tabs.items()}

    nc = bacc.Bacc("TRN2", target_bir_lowering=False, debug=False,
                   num_devices=N_CORES)
    dt = nc.dram_tensor
    io = (
        dt("xcc", [NSUPER, 128, 3, 4, SBATCH, N], F8,
           kind="ExternalInput").ap(),
        dt("lnmb", [BPC, 128, 4, 2, 2, N], F8, kind="ExternalInput").ap(),
        dt("wcat", [8, DIM, DIM], F8, kind="ExternalInput").ap(),
        dt("ident", [128, 128], BF16, kind="ExternalInput").ap(),
        dt("wc2", [DIM, DIM], BF16, kind="ExternalInput").ap(),
        dt("cvec", [4, DIM], F32, kind="ExternalInput").ap(),
        dt("outT", [BPC, DIM, N], BF16, kind="ExternalOutput").ap(),
    )
    with tile.TileContext(nc) as tc:
        _emit(nc, tc, io, n_super, ks)
    saved = bacc_mod.get_activation_tables
    bacc_mod.get_activation_tables = pinned_tables
    try:
        nc.compile()
    finally:
        bacc_mod.get_activation_tables = saved
    return nc


def _k_of(absmax, target=120.0):
    return int(math.floor(math.log2(target / max(absmax, 1e-30))))


def prep_in_maps(x, context, mask, Wq, Wk, Wv, Wc1, bc1, ln_g, ln_b, Wc2, bc2,
                 Wo, bo, bias_table, rel_index):
    f = np.float32
    x = np.asarray(x, f)
    context = np.asarray(context, f)
    mask = np.asarray(mask)
    Wq = np.asarray(Wq, f) * SCALE
    Wk = np.asarray(Wk, f)
    Wv = np.asarray(Wv, f)
    Wo = np.asarray(Wo, f)
    Wc1 = np.asarray(Wc1, f)
    Wc2 = np.asarray(Wc2, f)

    # q/k/v weights scaled 4 bits lower so fp8 activations (~sigma 29) stay
    # well inside IEEE e4m3 range (max finite 240)
    kq = _k_of(np.abs(Wq).max(), 7.5)
    kk = _k_of(np.abs(Wk).max(), 7.5)
    kv = _k_of(np.abs(Wv).max(), 7.5)
    kwo = _k_of(np.abs(Wo).max())
    kc1 = _k_of(np.abs(Wc1).max())
    w1sum = Wc1.sum(axis=1, keepdims=True)
    ksum = _k_of(np.abs(w1sum).max())
    lng_a = np.asarray(ln_g, f)
    lnb_a = np.asarray(ln_b, f)
    guni = (float(lng_a[0]) if np.all(lng_a == lng_a[0]) and float(lng_a[0]) > 0
            and np.all(lnb_a == 0.0) else None)
    b1z = bool(np.all(np.asarray(bc1, f) == 0.0))
    boz = bool(np.all(np.asarray(bo, f) == 0.0)
               and np.all(np.asarray(bc2, f) == 0.0))
    ks = (kq, kk, kv, kwo, kc1, ksum, guni, b1z, boz)
    g = OT_K + kwo

    xT = np.ascontiguousarray(
        x.reshape(N_CORES, BPC, N, DIM).transpose(0, 1, 3, 2))
    cT = np.ascontiguousarray(
        context.reshape(N_CORES, BPC, N, DIM).transpose(0, 1, 3, 2))
    x8 = xT.astype(NP8)
    c8 = cT.astype(NP8)
    clo = ((cT - c8.astype(f)) * 32.0).astype(NP8)

    def slab(a):
        # [cr, BPC, 512, 256] -> [cr, NSUPER, 128(p), 4(kc), SBATCH(j), 256]
        a = a.reshape(N_CORES, NSUPER, SBATCH, 4, 128, N)
        return a.transpose(0, 1, 4, 3, 2, 5)

    # one slab: [cr, NSUPER, 128, 3(x/c/clo), 4, SBATCH, 256]
    xcc = np.ascontiguousarray(
        np.stack([slab(x8), slab(c8), slab(clo)], axis=3))

    # ln(mask * expB):  [cr, b, 128(p), 4(c), 2(h2), 2(mc), 256(n)] fp8
    lnbias = np.asarray(bias_table, f)[np.asarray(rel_index)].transpose(2, 1, 0)
    mT = mask.reshape(N_CORES, BPC, N, N).transpose(0, 1, 3, 2)
    lnmb = np.where(mT[:, :, None, :, :] == 0, np.float32(MASKLN),
                    lnbias[None, None].astype(f))
    lnmb = lnmb.reshape(N_CORES, BPC, 4, 2, 2, 128, N).transpose(
        0, 1, 5, 2, 3, 4, 6)
    lnmb = np.ascontiguousarray(lnmb).astype(NP8)

    w1as = Wc1 * 2.0 ** kc1
    w1a = w1as.astype(NP8)
    w1b = (w1as - w1a.astype(f)).astype(NP8)
    w1c = (w1as / 32.0).astype(NP8)
    w1s_pad = np.zeros((DIM, DIM), f)
    w1s_pad[:, 0:8] = np.repeat(w1sum * 2.0 ** ksum, 8, axis=1)

    wcat = np.ascontiguousarray(np.stack([
        (Wq * 2.0 ** kq).astype(NP8),
        (Wk * 2.0 ** kk).astype(NP8),
        (Wv * 2.0 ** kv).astype(NP8),
        (Wo * 2.0 ** kwo).astype(NP8),
        w1a, w1b, w1c,
        w1s_pad.astype(NP8),
    ]))

    ident = np.ascontiguousarray(
        (np.eye(128, dtype=f) * 2.0 ** (kq + kk)).astype(NPBF))
    cvec = np.ascontiguousarray(np.stack([
        np.asarray(ln_g, f) * math.sqrt(DIM),
        np.asarray(ln_b, f),
        np.asarray(bc1, f),
        np.asarray(bo, f) + np.asarray(bc2, f),
    ]))

    shared = dict(
        wcat=wcat,
        ident=ident,
        wc2=np.ascontiguousarray(Wc2 * 2.0 ** g).astype(NPBF),
        cvec=cvec,
    )
    in_maps = [dict(xcc=xcc[c], lnmb=lnmb[c], **shared)
               for c in range(N_CORES)]
    return in_maps, ks


_nc_cache = {}


def _get_nc(n_super, ks):
    key = (n_super, ks)
    if key not in _nc_cache:
        _nc_cache[key] = build(n_super, ks)
    return _nc_cache[key]


def assemble_out(results):
    outT = np.stack([np.asarray(results[c]["outT"]).astype(np.float32)
                     for c in range(N_CORES)])
    return np.ascontiguousarray(
        outT.transpose(0, 1, 3, 2).reshape(B, N, DIM))


def kernel(**inputs):
    in_maps, ks = prep_in_maps(**inputs)
    nc = _get_nc(NSUPER, ks)
    res = run_bass_kernel_spmd(nc, in_maps, core_ids=list(range(N_CORES)))
    return assemble_out(res.results)
